# revision 69
# baseline (speedup 1.0000x reference)
"""Trainium2 Bass kernel for nn_Attention_81793357185069.

4-group attention: N=16, L=M=1024, in/param dim 512, planes 512, out 2048.
Strategy: data-parallel over batch N across 8 NeuronCores (2 batches/core),
zero collectives. All matmuls run in bf16 (1 cycle/row on PE vs 4 for fp32)
with fp32 PSUM accumulation.

Layout trick: everything is computed in "transposed" space so the PE
contraction dim always lands on partitions with zero on-chip transposes:
  - host pre-transposes activations X -> X^T (c, t) and weights W -> W^T
  - Q^T/K^T (planes, t) and V (t, planes) come straight out of projections
  - scores S^T (l, m) = (K^T_g).T @ Q^T_g per group
  - softmax has no max-subtraction (scores are bounded ~ N(0, 0.5^2)) and
    normalization is deferred: P = exp(S^T), denominator j-tiles are
    tree-summed on DVE (bf16, 2x mode) and the final 128-partition
    reduction runs on the otherwise-idle Pool/GPSIMD engine
    (partition_all_reduce) — zero PE matmuls for the denominator in 7 of
    8 units (the last unit keeps a 2-matmul ones-reduction on PE because
    its denom chain gates the b1-proj epilogue)
  - sv (d, m) = V_g.T @ P accumulated over l-tiles, scaled by 1/denom on
    PSUM evacuation
  - out (m, o) = sv_all.T @ Wproj^T; the scale 1/sqrt(512), ATTN_SCALE and
    all biases are folded in host-side (v_b/proj_b fold into a constant
    output vector since sum_l sim = 1).

Scheduling: each (batch, group) attention unit is woven with "filler"
matmul work — the next batch's QKV projections or the previous batch's
output projection — so the in-order PE stream never stalls on ACT exp
throughput or on the DVE denominator tree (the mid fillers sit between
the last S matmul and the SV tail, buying time for the tree before the
reduction consumes it). Engine balance: ACT carries all exps (one
1024-wide exp per S j-step from a 2-bank PSUM tile) + prologue/epilogue
evacs; DVE carries the denominator tree, recip/svmul, and the
woven-filler evacs; Pool carries the denominator partition-reductions.
DMAs are few and large (one 3-dim-AP DMA per input tensor via
AP.rearrange; one merged fp16 output DMA per 128-token row-block)
because per-DMA queue overhead (~1us) dominates with many small
transfers. ~3us of dummy PE matmuls during the initial DMA wait keep
the HAM clock-gate warm for the first real matmuls.
"""

import math

import ml_dtypes
import numpy as np

import concourse.bass as bass
import concourse.mybir as mybir
import concourse.tile as tile
from concourse import bacc, bass_isa
from concourse.bass_utils import run_bass_kernel_spmd

N_CORES = 8
N = 16
B = N // N_CORES  # batches per core
T = 1024  # L == M
C = 512  # in/param dim
P = 512  # planes
O = 2048  # out dim
G = 4  # groups
D = P // G  # 128 group planes
ATTN_SCALE = P ** (-0.5)
EQ_SCALE = 1.0 / math.sqrt(C)

CT = C // 128  # 4 contraction tiles
TT = T // 128  # 8 l/m tiles
MCH = T // 512  # 2 moving chunks of 512

BF = mybir.dt.bfloat16
F16 = mybir.dt.float16
F32 = mybir.dt.float32

_CACHE: dict = {}


def _emit(tc, has_cout):
    nc = tc.nc
    AF = mybir.ActivationFunctionType

    xt_op = nc.dram_tensor("xt_op", [B, C, T], BF, kind="ExternalInput").ap()
    xt_att = nc.dram_tensor("xt_att", [B, C, T], BF, kind="ExternalInput").ap()
    wqT = nc.dram_tensor("wqT", [C, P], BF, kind="ExternalInput").ap()
    wkT = nc.dram_tensor("wkT", [C, P], BF, kind="ExternalInput").ap()
    wvT = nc.dram_tensor("wvT", [C, P], BF, kind="ExternalInput").ap()
    wpT = nc.dram_tensor("wpT", [P, O], BF, kind="ExternalInput").ap()
    qb = nc.dram_tensor("qb", [P, 1], F32, kind="ExternalInput").ap()
    kb = nc.dram_tensor("kb", [P, 1], F32, kind="ExternalInput").ap()
    coutb = (
        nc.dram_tensor("coutb", [128, O], F32, kind="ExternalInput").ap()
        if has_cout
        else None
    )
    # fp16 device output: halves output-DMA bytes (the epilogue is
    # DMA-rate-limited with fp32), host casts back to fp32; adds ~1e-4
    # relative error
    out = nc.dram_tensor("out", [B, T, O], F16, kind="ExternalOutput").ap()

    with (
        tc.tile_pool(name="const", bufs=1) as const,
        tc.tile_pool(name="xt", bufs=1) as xtp,
        tc.tile_pool(name="qkv", bufs=2) as qkvp,
        tc.tile_pool(name="pt", bufs=2) as ptp,
        tc.tile_pool(name="tr", bufs=1) as trp,
        tc.tile_pool(name="accp", bufs=2) as accp,
        tc.tile_pool(name="rdn", bufs=2) as rdnp,
        tc.tile_pool(name="svt", bufs=2) as svtp,
        tc.tile_pool(name="ost", bufs=2) as ostp,
        tc.tile_pool(name="ps", bufs=2, space="PSUM") as psp,
        tc.tile_pool(name="sps", bufs=2, space="PSUM") as spsp,
        tc.tile_pool(name="pvp", bufs=1, space="PSUM") as pvp,
    ):
        # ---- constants / weights (one merged tile per tensor) ----
        # weight layout: [128, ct * P] — ct-tile i at columns [i*P, (i+1)*P)
        wq_s = const.tile([128, CT * P], BF, tag="wq", name="wq")
        wk_s = const.tile([128, CT * P], BF, tag="wk", name="wk")
        wv_s = const.tile([128, CT * P], BF, tag="wv", name="wv")
        wp_s = const.tile([128, CT * O], BF, tag="wp", name="wp")
        qb_s = const.tile([128, G], F32, tag="qb", name="qb")
        kb_s = const.tile([128, G], F32, tag="kb", name="kb")
        cout_s = const.tile([128, O], F32, tag="cout", name="cout") if has_cout else None
        ones_s = const.tile([128, 128], BF, tag="ones", name="ones")

        # activations: [128, ct * T]
        xo_s = {b: xtp.tile([128, CT * T], BF, tag=f"xo{b}", name=f"xo_{b}") for b in range(B)}
        xa_s = {b: xtp.tile([128, CT * T], BF, tag=f"xa{b}", name=f"xa_{b}") for b in range(B)}
        qT_s = {b: [qkvp.tile([128, T], BF, tag=f"q{g}", name=f"qT{g}_{b}") for g in range(G)] for b in range(B)}
        kT_s = {b: [qkvp.tile([128, T], BF, tag=f"k{g}", name=f"kT{g}_{b}") for g in range(G)] for b in range(B)}
        v_s = {b: [qkvp.tile([128, P], BF, tag=f"v{j}", name=f"v{j}_{b}") for j in range(TT)] for b in range(B)}
        svT = {b: [svtp.tile([128, T], BF, tag=f"s{g}", name=f"svT{g}_{b}") for g in range(G)] for b in range(B)}

        # ---- input DMAs: one per tensor, need-ordered, spread over both
        # HWDGE queues (scalar + sync) so bandwidth serves the earliest
        # consumer. Per-DMA queue overhead is ~1us, so few+large wins.
        xo0_src = xt_op[0].rearrange("(ct p) t -> p ct t", p=128)
        xa0_src = xt_att[0].rearrange("(ct p) t -> p ct t", p=128)
        # per-ct xo DMAs so the ct-outer first Q chunk can trickle-start
        # as tiles land; everything else one DMA per tensor in need order
        for i in range(CT):
            nc.scalar.dma_start(
                xo_s[0][:, i * T : (i + 1) * T], xo0_src[:, i : i + 1]
            )
        nc.sync.dma_start(wq_s[:], wqT.rearrange("(ct p) o -> p ct o", p=128))
        nc.sync.dma_start(qb_s[:], qb.rearrange("(g p) one -> p (g one)", p=128))
        nc.sync.dma_start(wk_s[:], wkT.rearrange("(ct p) o -> p ct o", p=128))
        nc.scalar.dma_start(xa_s[0][:, : 2 * T], xa0_src[:, :2])
        nc.scalar.dma_start(xa_s[0][:, 2 * T :], xa0_src[:, 2:])
        nc.sync.dma_start(kb_s[:], kb.rearrange("(g p) one -> p (g one)", p=128))
        nc.sync.dma_start(wv_s[:], wvT.rearrange("(ct p) o -> p ct o", p=128))
        nc.vector.memset(ones_s[:], 1.0)
        # preload the ACT exp table set during the idle head (the implicit
        # table load would otherwise stall the first critical exp by ~1.3us)
        expwarm = const.tile([128, 1], F32, tag="expwarm", name="expwarm")
        nc.scalar.activation(expwarm[:], ones_s[:, :1], AF.Exp)
        # PE warmup: ~3us of dummy matmuls during the initial DMA wait so
        # the HAM clock-gate (and the cost model's p-state ramp) reaches
        # full speed before the first real matmul; the result is never read
        warm = psp.tile([128, 512], F32, tag="ps", name="warm")
        for i in range(28):
            nc.tensor.matmul(
                warm[:, :128], ones_s[:], ones_s[:],
                start=(i == 0), stop=(i == 27),
            )
        nc.scalar.dma_start(
            xo_s[1][:], xt_op[1].rearrange("(ct p) t -> p ct t", p=128)
        )
        nc.scalar.dma_start(
            xa_s[1][:], xt_att[1].rearrange("(ct p) t -> p ct t", p=128)
        )
        nc.sync.dma_start(wp_s[:], wpT.rearrange("(ct p) o -> p ct o", p=128))
        if has_cout:
            nc.sync.dma_start(cout_s[:], coutb[:, :])

        # ---- chunk emitters (each: one PSUM group = 8 MMs + evacuation).
        # Evac engine rule: batch-0 QKV runs in the prologue (ACT idle →
        # ACT); batch-1 QKV is woven into b0 units whose ACT is saturated
        # with exps (→ DVE). b0 proj is woven into b1 units (→ DVE); b1
        # proj runs in the epilogue (→ ACT).
        def chunk_q(b, g, ct_outer=False):
            # ct_outer: first prologue chunk only — lets matmuls start as
            # soon as the first ct tile of xo lands (startup trickle)
            if ct_outer:
                pq = [
                    psp.tile([128, 512], F32, tag="ps", name=f"psq{g}{mch}_{b}")
                    for mch in range(MCH)
                ]
                for ct in range(CT):
                    for mch in range(MCH):
                        nc.tensor.matmul(
                            pq[mch][:],
                            wq_s[:, ct * P + g * 128 : ct * P + (g + 1) * 128],
                            xo_s[b][:, ct * T + mch * 512 : ct * T + (mch + 1) * 512],
                            start=(ct == 0),
                            stop=(ct == CT - 1),
                        )
                for mch in range(MCH):
                    osl = qT_s[b][g][:, mch * 512 : (mch + 1) * 512]
                    nc.scalar.activation(
                        osl, pq[mch][:], AF.Identity, bias=qb_s[:, g : g + 1]
                    )
                return
            for mch in range(MCH):
                pq = psp.tile([128, 512], F32, tag="ps", name=f"psq{g}{mch}_{b}")
                for ct in range(CT):
                    nc.tensor.matmul(
                        pq[:],
                        wq_s[:, ct * P + g * 128 : ct * P + (g + 1) * 128],
                        xo_s[b][:, ct * T + mch * 512 : ct * T + (mch + 1) * 512],
                        start=(ct == 0),
                        stop=(ct == CT - 1),
                    )
                osl = qT_s[b][g][:, mch * 512 : (mch + 1) * 512]
                if b == 0:
                    nc.scalar.activation(
                        osl, pq[:], AF.Identity, bias=qb_s[:, g : g + 1]
                    )
                else:
                    nc.vector.tensor_scalar_add(osl, pq[:], qb_s[:, g : g + 1])

        def chunk_k(b, g):
            for mch in range(MCH):
                pk = psp.tile([128, 512], F32, tag="ps", name=f"psk{g}{mch}_{b}")
                for ct in range(CT):
                    nc.tensor.matmul(
                        pk[:],
                        wk_s[:, ct * P + g * 128 : ct * P + (g + 1) * 128],
                        xa_s[b][:, ct * T + mch * 512 : ct * T + (mch + 1) * 512],
                        start=(ct == 0),
                        stop=(ct == CT - 1),
                    )
                osl = kT_s[b][g][:, mch * 512 : (mch + 1) * 512]
                if b == 0:
                    nc.scalar.activation(
                        osl, pk[:], AF.Identity, bias=kb_s[:, g : g + 1]
                    )
                else:
                    nc.vector.tensor_scalar_add(osl, pk[:], kb_s[:, g : g + 1])

        def chunk_v(b, jj):
            for jh in range(2):
                j = jj * 2 + jh
                pv = psp.tile([128, 512], F32, tag="ps", name=f"psv{j}_{b}")
                for ct in range(CT):
                    nc.tensor.matmul(
                        pv[:],
                        xa_s[b][:, ct * T + j * 128 : ct * T + (j + 1) * 128],
                        wv_s[:, ct * P : (ct + 1) * P],
                        start=(ct == 0),
                        stop=(ct == CT - 1),
                    )
                if b == 0:
                    nc.scalar.copy(v_s[b][j][:], pv[:])
                else:
                    nc.vector.tensor_copy(v_s[b][j][:], pv[:])

        ost_live = {}

        def chunk_proj(b, mt, oh, evac_act=False):
            # ost holds both oh halves of an mt row-block; one merged
            # output DMA per mt fires when the oh=1 half is evacuated
            if oh == 0:
                ost_live[(b, mt)] = ostp.tile(
                    [128, O], F16, tag="ost", name=f"ost{mt}_{b}"
                )
            ost = ost_live[(b, mt)]
            for oc in range(2):
                po = psp.tile([128, 512], F32, tag="ps", name=f"pso{mt}{oh}{oc}_{b}")
                for g in range(G):
                    nc.tensor.matmul(
                        po[:],
                        svT[b][g][:, mt * 128 : (mt + 1) * 128],
                        wp_s[:, g * O + oh * 1024 + oc * 512 : g * O + oh * 1024 + (oc + 1) * 512],
                        start=(g == 0),
                        stop=(g == G - 1),
                    )
                osl = ost[:, oh * 1024 + oc * 512 : oh * 1024 + (oc + 1) * 512]
                if has_cout:
                    nc.vector.tensor_add(
                        osl, po[:],
                        cout_s[:, oh * 1024 + oc * 512 : oh * 1024 + (oc + 1) * 512],
                    )
                elif b == 0 and not evac_act:
                    nc.vector.tensor_copy(osl, po[:])
                else:
                    nc.scalar.copy(osl, po[:])
            if b == 1 and mt == TT - 1:
                # last row-block: fire per-oh halves so the final DMA (and
                # the kernel tail behind it) is half as long
                nc.sync.dma_start(
                    out[b, mt * 128 : (mt + 1) * 128, oh * 1024 : (oh + 1) * 1024],
                    ost[:, oh * 1024 : (oh + 1) * 1024],
                )
                if oh == 1:
                    del ost_live[(b, mt)]
            elif oh == 1:
                nc.sync.dma_start(out[b, mt * 128 : (mt + 1) * 128, :], ost[:])
                del ost_live[(b, mt)]

        # ---- attention unit with woven filler chunks ----
        def emit_unit(b, g, early=(), mid=(), post=()):
            """S^T+exp for (b,g), SV lagged, DVE denom tree, fillers woven.

            PE stream: S0 S1 [early] S2 S3 S4+sv0 S5+sv1 S6+sv2 S7+sv3
                       [mid] sv4..sv7 den0 den1, recip+svmul (DVE), [post].
            DVE stream (per unit): t0..t3 (pair sums of exp tiles, bf16 2x),
            u0 u1 acc (tree), recip+svmul. The mid fillers buy time for
            exp7 + the tree tail before the PE hits den0.
            """
            pT = [ptp.tile([128, T], BF, tag=f"p{j}", name=f"pT{j}_{g}_{b}") for j in range(TT)]
            pv2 = pvp.tile([128, T], F32, tag="pv", name=f"pssv_{g}_{b}")
            tt = [trp.tile([128, T], BF, tag=f"t{i}", name=f"t{i}_{g}_{b}") for i in range(4)]
            uu = [trp.tile([128, T], BF, tag=f"u{i}", name=f"u{i}_{g}_{b}") for i in range(2)]
            acc = accp.tile([128, T], BF, tag="acc", name=f"acc_{g}_{b}")

            def s_step(j):
                # one 2-bank PSUM tile per j: both mch matmuls land in it,
                # then a single 1024-wide exp evacuates it (halves ACT's
                # per-instruction init overhead vs two 512-wide exps)
                ps = spsp.tile([128, T], F32, tag="sps", name=f"pss{j}_{g}_{b}")
                for mch in range(MCH):
                    nc.tensor.matmul(
                        ps[:, mch * 512 : (mch + 1) * 512],
                        kT_s[b][g][:, j * 128 : (j + 1) * 128],
                        qT_s[b][g][:, mch * 512 : (mch + 1) * 512],
                        start=True,
                        stop=True,
                    )
                nc.scalar.activation(pT[j][:], ps[:], AF.Exp)

            def sv_step(j):
                for mch in range(MCH):
                    nc.tensor.matmul(
                        pv2[:, mch * 512 : (mch + 1) * 512],
                        v_s[b][j][:, g * 128 : (g + 1) * 128],
                        pT[j][:, mch * 512 : (mch + 1) * 512],
                        start=(j == 0),
                        stop=(j == TT - 1),
                    )

            s_step(0)
            s_step(1)
            nc.vector.tensor_add(tt[0][:], pT[0][:], pT[1][:])
            for f in early:
                f()
            s_step(2)
            s_step(3)
            nc.vector.tensor_add(tt[1][:], pT[2][:], pT[3][:])
            nc.vector.tensor_add(uu[0][:], tt[0][:], tt[1][:])
            for j in range(4, TT):
                s_step(j)
                sv_step(j - 4)
                if j == 5:
                    nc.vector.tensor_add(tt[2][:], pT[4][:], pT[5][:])
                if j == 7:
                    nc.vector.tensor_add(tt[3][:], pT[6][:], pT[7][:])
                    nc.vector.tensor_add(uu[1][:], tt[2][:], tt[3][:])
                    nc.vector.tensor_add(acc[:], uu[0][:], uu[1][:])
            for f in mid:
                f()
            for j in range(TT - 4, TT):
                sv_step(j)
            # denominator partition-reduction: on the (otherwise idle) Pool
            # engine for most units; the last unit keeps the PE ones-matmul
            # because its denom→recip→svmul chain gates the whole b1-proj
            # epilogue and the Q7 software op's latency is less predictable
            # denominator partition-reduction: on the (otherwise idle) Pool
            # engine for most units; the last unit keeps the PE ones-matmul
            # because its denom→recip→svmul chain gates the whole b1-proj
            # epilogue and the Q7 software op's latency is less predictable
            if (b, g) != (1, 3):
                pd = rdnp.tile([128, T], F32, tag="pd", name=f"pd{g}_{b}")
                nc.gpsimd.partition_all_reduce(
                    pd[:], acc[:], channels=128, reduce_op=bass_isa.ReduceOp.add
                )
            else:
                pd = spsp.tile([128, T], F32, tag="sps", name=f"psd_{g}_{b}")
                for m in range(MCH):
                    nc.tensor.matmul(
                        pd[:, m * 512 : (m + 1) * 512],
                        ones_s[:],
                        acc[:, m * 512 : (m + 1) * 512],
                        start=True,
                        stop=True,
                    )
            rden = rdnp.tile([128, T], F32, tag="rd", name=f"rden{g}_{b}")
            nc.vector.reciprocal(rden[:], pd[:])
            nc.vector.tensor_mul(svT[b][g][:], pv2[:], rden[:])
            for f in post:
                f()

        # ---- whole-kernel schedule ----
        # prologue: batch-0 QKV — all Q first (its inputs stream in first),
        # then K, then V
        chunk_q(0, 0, ct_outer=True)
        for g in range(1, G):
            chunk_q(0, g)
        for g in range(G):
            chunk_k(0, g)
        for jj in range(TT // 2):
            chunk_v(0, jj)

        # batch-0 units carry batch-1 QKV as filler;
        # batch-1 units carry batch-0 proj as filler.
        def fq(g):
            return lambda: chunk_q(1, g)

        def fk(g):
            return lambda: chunk_k(1, g)

        def fv(jj):
            return lambda: chunk_v(1, jj)

        def fp(b, a, evac_act=False):
            return lambda: chunk_proj(b, a // 2, a % 2, evac_act=evac_act)

        emit_unit(0, 0, early=[fq(0)], mid=[fq(1)], post=[fq(2)])
        emit_unit(0, 1, early=[fq(3)], mid=[fk(0)], post=[fk(1)])
        emit_unit(0, 2, early=[fk(2)], mid=[fk(3)], post=[fv(0)])
        emit_unit(0, 3, early=[fv(1)], mid=[fv(2)], post=[fv(3)])
        # proj(0) needs svT(0,3), which lands during unit (1,0): no early
        # there. unit (1,3) gets a multi-chunk post so the PE has work
        # while its own DVE tail (tree→den→recip→svmul for svT(1,3))
        # retires — the entire b1 proj epilogue waits on that tail.
        emit_unit(1, 0, mid=[fp(0, 0), fp(0, 1)], post=[fp(0, 2)])
        emit_unit(1, 1, early=[fp(0, 3)], mid=[fp(0, 4)], post=[fp(0, 5)])
        emit_unit(1, 2, early=[fp(0, 6), fp(0, 7)], mid=[fp(0, 8)], post=[fp(0, 9)])
        # unit (1,3): filler evacs on ACT — its DVE chain (tree → recip →
        # svmul for svT(1,3)) gates the entire b1 proj epilogue
        emit_unit(1, 3, early=[fp(0, 10, True), fp(0, 11, True)],
                  mid=[fp(0, 12, True)],
                  post=[fp(0, 13, True), fp(0, 14, True), fp(0, 15, True)])
        # epilogue: batch-1 proj
        for a in range(16):
            chunk_proj(1, a // 2, a % 2)


def _build(has_cout):
    nc = bacc.Bacc(
        "TRN2", target_bir_lowering=False, debug=False, num_devices=N_CORES
    )
    with tile.TileContext(nc) as tc:
        _emit(tc, has_cout)
    nc.compile()
    return nc


def get_nc(has_cout=False):
    key = ("nc", has_cout)
    if key not in _CACHE:
        _CACHE[key] = _build(has_cout)
    return _CACHE[key]


def prep_inputs(attention, op_param, q_w, q_b, k_w, k_b, v_w, v_b, proj_w, proj_b):
    """Host-side layout prep: fold scales, transpose, cast to bf16, shard."""
    bf16 = ml_dtypes.bfloat16
    f32 = np.float32

    att = np.asarray(attention, f32)
    op = np.asarray(op_param, f32)

    # (n, t, c) -> (n, c, t), bf16
    xt_att = np.ascontiguousarray(att.transpose(0, 2, 1)).astype(bf16)
    xt_op = np.ascontiguousarray(op.transpose(0, 2, 1)).astype(bf16)

    wqT = np.ascontiguousarray(
        (np.asarray(q_w, f32) * (EQ_SCALE * ATTN_SCALE)).T
    ).astype(bf16)
    wkT = np.ascontiguousarray((np.asarray(k_w, f32) * EQ_SCALE).T).astype(bf16)
    wvT = np.ascontiguousarray((np.asarray(v_w, f32) * EQ_SCALE).T).astype(bf16)
    # proj is also an EqualLinear: weight scale 1/sqrt(PLANES) = EQ_SCALE
    wp_scaled = np.asarray(proj_w, f32) * EQ_SCALE
    wpT = np.ascontiguousarray(wp_scaled.T).astype(bf16)

    qb2 = (np.asarray(q_b, f32) * ATTN_SCALE).reshape(P, 1)
    kb2 = np.asarray(k_b, f32).reshape(P, 1)
    # sum_l sim = 1, so v_b contributes proj_w @ v_b to every output row
    cout = wp_scaled @ np.asarray(v_b, f32) + np.asarray(proj_b, f32)
    has_cout = bool(np.any(cout != 0.0))
    coutb = np.ascontiguousarray(np.broadcast_to(cout[None, :], (128, O))).astype(f32)

    in_maps = []
    for core in range(N_CORES):
        lo, hi = core * B, (core + 1) * B
        m = {
            "xt_op": np.ascontiguousarray(xt_op[lo:hi]),
            "xt_att": np.ascontiguousarray(xt_att[lo:hi]),
            "wqT": wqT,
            "wkT": wkT,
            "wvT": wvT,
            "wpT": wpT,
            "qb": qb2,
            "kb": kb2,
        }
        if has_cout:
            m["coutb"] = coutb
        in_maps.append(m)
    return in_maps


def run(in_maps, trace=False, **kw):
    has_cout = "coutb" in in_maps[0]
    nc = get_nc(has_cout)
    res = run_bass_kernel_spmd(nc, in_maps, list(range(N_CORES)), trace=trace, **kw)
    return res


def kernel(**inputs) -> np.ndarray:
    in_maps = prep_inputs(**inputs)
    res = run(in_maps)
    out = np.concatenate([res.results[i]["out"] for i in range(N_CORES)], axis=0)
    return out.astype(np.float32)


# revision 74
# speedup vs baseline: 1.0022x; 1.0022x over previous
"""Trainium2 Bass kernel for nn_Attention_81793357185069.

4-group attention: N=16, L=M=1024, in/param dim 512, planes 512, out 2048.
Strategy: data-parallel over batch N across 8 NeuronCores (2 batches/core),
zero collectives. All matmuls run in bf16 (1 cycle/row on PE vs 4 for fp32)
with fp32 PSUM accumulation.

Layout trick: everything is computed in "transposed" space so the PE
contraction dim always lands on partitions with zero on-chip transposes:
  - host pre-transposes activations X -> X^T (c, t) and weights W -> W^T
  - Q^T/K^T (planes, t) and V (t, planes) come straight out of projections
  - scores S^T (l, m) = (K^T_g).T @ Q^T_g per group
  - softmax has no max-subtraction (scores are bounded ~ N(0, 0.5^2)) and
    normalization is deferred: P = exp(S^T), denominator j-tiles are
    tree-summed on DVE (bf16, 2x mode) and the final 128-partition
    reduction runs on the otherwise-idle Pool/GPSIMD engine
    (partition_all_reduce) — zero PE matmuls for the denominator in 7 of
    8 units (the last unit keeps a 2-matmul ones-reduction on PE because
    its denom chain gates the b1-proj epilogue)
  - sv (d, m) = V_g.T @ P accumulated over l-tiles, scaled by 1/denom on
    PSUM evacuation
  - out (m, o) = sv_all.T @ Wproj^T; the scale 1/sqrt(512), ATTN_SCALE and
    all biases are folded in host-side (v_b/proj_b fold into a constant
    output vector since sum_l sim = 1).

Scheduling: each (batch, group) attention unit is woven with "filler"
matmul work — the next batch's QKV projections or the previous batch's
output projection — so the in-order PE stream never stalls on ACT exp
throughput or on the DVE denominator tree (the mid fillers sit between
the last S matmul and the SV tail, buying time for the tree before the
reduction consumes it). Engine balance: ACT carries all exps (one
1024-wide exp per S j-step from a 2-bank PSUM tile) + prologue/epilogue
evacs; DVE carries the denominator tree, recip/svmul, and the
woven-filler evacs; Pool carries the denominator partition-reductions.
DMAs are few and large (one 3-dim-AP DMA per input tensor via
AP.rearrange; one merged fp16 output DMA per 128-token row-block)
because per-DMA queue overhead (~1us) dominates with many small
transfers. ~3us of dummy PE matmuls during the initial DMA wait keep
the HAM clock-gate warm for the first real matmuls.
"""

import math

import ml_dtypes
import numpy as np

import concourse.bass as bass
import concourse.mybir as mybir
import concourse.tile as tile
from concourse import bacc, bass_isa
from concourse.bass_utils import run_bass_kernel_spmd

N_CORES = 8
N = 16
B = N // N_CORES  # batches per core
T = 1024  # L == M
C = 512  # in/param dim
P = 512  # planes
O = 2048  # out dim
G = 4  # groups
D = P // G  # 128 group planes
ATTN_SCALE = P ** (-0.5)
EQ_SCALE = 1.0 / math.sqrt(C)

CT = C // 128  # 4 contraction tiles
TT = T // 128  # 8 l/m tiles
MCH = T // 512  # 2 moving chunks of 512

BF = mybir.dt.bfloat16
F16 = mybir.dt.float16
F32 = mybir.dt.float32

_CACHE: dict = {}


def _emit(tc, has_cout):
    nc = tc.nc
    AF = mybir.ActivationFunctionType

    xt_op = nc.dram_tensor("xt_op", [B, C, T], BF, kind="ExternalInput").ap()
    xt_att = nc.dram_tensor("xt_att", [B, C, T], BF, kind="ExternalInput").ap()
    wqT = nc.dram_tensor("wqT", [C, P], BF, kind="ExternalInput").ap()
    wkT = nc.dram_tensor("wkT", [C, P], BF, kind="ExternalInput").ap()
    wvT = nc.dram_tensor("wvT", [C, P], BF, kind="ExternalInput").ap()
    wpT = nc.dram_tensor("wpT", [P, O], BF, kind="ExternalInput").ap()
    qb = nc.dram_tensor("qb", [P, 1], F32, kind="ExternalInput").ap()
    kb = nc.dram_tensor("kb", [P, 1], F32, kind="ExternalInput").ap()
    coutb = (
        nc.dram_tensor("coutb", [128, O], F32, kind="ExternalInput").ap()
        if has_cout
        else None
    )
    # fp16 device output: halves output-DMA bytes (the epilogue is
    # DMA-rate-limited with fp32), host casts back to fp32; adds ~1e-4
    # relative error
    out = nc.dram_tensor("out", [B, T, O], F16, kind="ExternalOutput").ap()

    with (
        tc.tile_pool(name="const", bufs=1) as const,
        tc.tile_pool(name="xt", bufs=1) as xtp,
        tc.tile_pool(name="qkv", bufs=2) as qkvp,
        tc.tile_pool(name="pt", bufs=2) as ptp,
        tc.tile_pool(name="tr", bufs=1) as trp,
        tc.tile_pool(name="accp", bufs=2) as accp,
        tc.tile_pool(name="rdn", bufs=2) as rdnp,
        tc.tile_pool(name="svt", bufs=2) as svtp,
        tc.tile_pool(name="ost", bufs=2) as ostp,
        tc.tile_pool(name="ps", bufs=2, space="PSUM") as psp,
        tc.tile_pool(name="sps", bufs=2, space="PSUM") as spsp,
        tc.tile_pool(name="pvp", bufs=1, space="PSUM") as pvp,
    ):
        # ---- constants / weights (one merged tile per tensor) ----
        # weight layout: [128, ct * P] — ct-tile i at columns [i*P, (i+1)*P)
        wq_s = const.tile([128, CT * P], BF, tag="wq", name="wq")
        wk_s = const.tile([128, CT * P], BF, tag="wk", name="wk")
        wv_s = const.tile([128, CT * P], BF, tag="wv", name="wv")
        wp_s = const.tile([128, CT * O], BF, tag="wp", name="wp")
        qb_s = const.tile([128, G], F32, tag="qb", name="qb")
        kb_s = const.tile([128, G], F32, tag="kb", name="kb")
        cout_s = const.tile([128, O], F32, tag="cout", name="cout") if has_cout else None
        ones_s = const.tile([128, 128], BF, tag="ones", name="ones")

        # activations: [128, ct * T]
        xo_s = {b: xtp.tile([128, CT * T], BF, tag=f"xo{b}", name=f"xo_{b}") for b in range(B)}
        xa_s = {b: xtp.tile([128, CT * T], BF, tag=f"xa{b}", name=f"xa_{b}") for b in range(B)}
        qT_s = {b: [qkvp.tile([128, T], BF, tag=f"q{g}", name=f"qT{g}_{b}") for g in range(G)] for b in range(B)}
        kT_s = {b: [qkvp.tile([128, T], BF, tag=f"k{g}", name=f"kT{g}_{b}") for g in range(G)] for b in range(B)}
        v_s = {b: [qkvp.tile([128, P], BF, tag=f"v{j}", name=f"v{j}_{b}") for j in range(TT)] for b in range(B)}
        svT = {b: [svtp.tile([128, T], BF, tag=f"s{g}", name=f"svT{g}_{b}") for g in range(G)] for b in range(B)}

        # ---- input DMAs: one per tensor, need-ordered, spread over both
        # HWDGE queues (scalar + sync) so bandwidth serves the earliest
        # consumer. Per-DMA queue overhead is ~1us, so few+large wins.
        xo0_src = xt_op[0].rearrange("(ct p) t -> p ct t", p=128)
        xa0_src = xt_att[0].rearrange("(ct p) t -> p ct t", p=128)
        # per-ct xo DMAs so the ct-outer first Q chunk can trickle-start
        # as tiles land; everything else one DMA per tensor in need order
        for i in range(CT):
            nc.scalar.dma_start(
                xo_s[0][:, i * T : (i + 1) * T], xo0_src[:, i : i + 1]
            )
        nc.sync.dma_start(wq_s[:], wqT.rearrange("(ct p) o -> p ct o", p=128))
        nc.sync.dma_start(qb_s[:], qb.rearrange("(g p) one -> p (g one)", p=128))
        nc.sync.dma_start(wk_s[:], wkT.rearrange("(ct p) o -> p ct o", p=128))
        nc.scalar.dma_start(xa_s[0][:, : 2 * T], xa0_src[:, :2])
        nc.scalar.dma_start(xa_s[0][:, 2 * T :], xa0_src[:, 2:])
        nc.sync.dma_start(kb_s[:], kb.rearrange("(g p) one -> p (g one)", p=128))
        nc.sync.dma_start(wv_s[:], wvT.rearrange("(ct p) o -> p ct o", p=128))
        nc.vector.memset(ones_s[:], 1.0)
        # preload the ACT exp table set during the idle head (the implicit
        # table load would otherwise stall the first critical exp by ~1.3us)
        expwarm = const.tile([128, 1], F32, tag="expwarm", name="expwarm")
        nc.scalar.activation(expwarm[:], ones_s[:, :1], AF.Exp)
        # PE warmup: ~3us of dummy matmuls during the initial DMA wait so
        # the HAM clock-gate (and the cost model's p-state ramp) reaches
        # full speed before the first real matmul; the result is never read
        warm = psp.tile([128, 512], F32, tag="ps", name="warm")
        for i in range(28):
            nc.tensor.matmul(
                warm[:, :128], ones_s[:], ones_s[:],
                start=(i == 0), stop=(i == 27),
            )
        nc.scalar.dma_start(
            xo_s[1][:], xt_op[1].rearrange("(ct p) t -> p ct t", p=128)
        )
        nc.scalar.dma_start(
            xa_s[1][:], xt_att[1].rearrange("(ct p) t -> p ct t", p=128)
        )
        nc.sync.dma_start(wp_s[:], wpT.rearrange("(ct p) o -> p ct o", p=128))
        if has_cout:
            nc.sync.dma_start(cout_s[:], coutb[:, :])

        # ---- chunk emitters (each: one PSUM group = 8 MMs + evacuation).
        # Evac engine rule: batch-0 QKV runs in the prologue (ACT idle →
        # ACT); batch-1 QKV is woven into b0 units whose ACT is saturated
        # with exps (→ DVE). b0 proj is woven into b1 units (→ DVE); b1
        # proj runs in the epilogue (→ ACT).
        def chunk_q(b, g, ct_outer=False):
            # ct_outer: first prologue chunk only — lets matmuls start as
            # soon as the first ct tile of xo lands (startup trickle)
            if ct_outer:
                pq = [
                    psp.tile([128, 512], F32, tag="ps", name=f"psq{g}{mch}_{b}")
                    for mch in range(MCH)
                ]
                for ct in range(CT):
                    for mch in range(MCH):
                        nc.tensor.matmul(
                            pq[mch][:],
                            wq_s[:, ct * P + g * 128 : ct * P + (g + 1) * 128],
                            xo_s[b][:, ct * T + mch * 512 : ct * T + (mch + 1) * 512],
                            start=(ct == 0),
                            stop=(ct == CT - 1),
                        )
                for mch in range(MCH):
                    osl = qT_s[b][g][:, mch * 512 : (mch + 1) * 512]
                    nc.scalar.activation(
                        osl, pq[mch][:], AF.Identity, bias=qb_s[:, g : g + 1]
                    )
                return
            for mch in range(MCH):
                pq = psp.tile([128, 512], F32, tag="ps", name=f"psq{g}{mch}_{b}")
                for ct in range(CT):
                    nc.tensor.matmul(
                        pq[:],
                        wq_s[:, ct * P + g * 128 : ct * P + (g + 1) * 128],
                        xo_s[b][:, ct * T + mch * 512 : ct * T + (mch + 1) * 512],
                        start=(ct == 0),
                        stop=(ct == CT - 1),
                    )
                osl = qT_s[b][g][:, mch * 512 : (mch + 1) * 512]
                if b == 0:
                    nc.scalar.activation(
                        osl, pq[:], AF.Identity, bias=qb_s[:, g : g + 1]
                    )
                else:
                    nc.vector.tensor_scalar_add(osl, pq[:], qb_s[:, g : g + 1])

        def chunk_k(b, g):
            for mch in range(MCH):
                pk = psp.tile([128, 512], F32, tag="ps", name=f"psk{g}{mch}_{b}")
                for ct in range(CT):
                    nc.tensor.matmul(
                        pk[:],
                        wk_s[:, ct * P + g * 128 : ct * P + (g + 1) * 128],
                        xa_s[b][:, ct * T + mch * 512 : ct * T + (mch + 1) * 512],
                        start=(ct == 0),
                        stop=(ct == CT - 1),
                    )
                osl = kT_s[b][g][:, mch * 512 : (mch + 1) * 512]
                if b == 0:
                    nc.scalar.activation(
                        osl, pk[:], AF.Identity, bias=kb_s[:, g : g + 1]
                    )
                else:
                    nc.vector.tensor_scalar_add(osl, pk[:], kb_s[:, g : g + 1])

        def chunk_v(b, jj):
            for jh in range(2):
                j = jj * 2 + jh
                pv = psp.tile([128, 512], F32, tag="ps", name=f"psv{j}_{b}")
                for ct in range(CT):
                    nc.tensor.matmul(
                        pv[:],
                        xa_s[b][:, ct * T + j * 128 : ct * T + (j + 1) * 128],
                        wv_s[:, ct * P : (ct + 1) * P],
                        start=(ct == 0),
                        stop=(ct == CT - 1),
                    )
                if b == 0:
                    nc.scalar.copy(v_s[b][j][:], pv[:])
                else:
                    nc.vector.tensor_copy(v_s[b][j][:], pv[:])

        ost_live = {}

        def chunk_proj(b, mt, oh, evac_act=False):
            # ost holds both oh halves of an mt row-block; one merged
            # output DMA per mt fires when the oh=1 half is evacuated
            if oh == 0:
                ost_live[(b, mt)] = ostp.tile(
                    [128, O], F16, tag="ost", name=f"ost{mt}_{b}"
                )
            ost = ost_live[(b, mt)]
            for oc in range(2):
                po = psp.tile([128, 512], F32, tag="ps", name=f"pso{mt}{oh}{oc}_{b}")
                for g in range(G):
                    nc.tensor.matmul(
                        po[:],
                        svT[b][g][:, mt * 128 : (mt + 1) * 128],
                        wp_s[:, g * O + oh * 1024 + oc * 512 : g * O + oh * 1024 + (oc + 1) * 512],
                        start=(g == 0),
                        stop=(g == G - 1),
                    )
                osl = ost[:, oh * 1024 + oc * 512 : oh * 1024 + (oc + 1) * 512]
                if has_cout:
                    nc.vector.tensor_add(
                        osl, po[:],
                        cout_s[:, oh * 1024 + oc * 512 : oh * 1024 + (oc + 1) * 512],
                    )
                elif b == 0 and not evac_act:
                    nc.vector.tensor_copy(osl, po[:])
                else:
                    nc.scalar.copy(osl, po[:])
            if b == 1 and mt == TT - 1:
                # last row-block: fire per-oh halves so the final DMA (and
                # the kernel tail behind it) is half as long
                nc.sync.dma_start(
                    out[b, mt * 128 : (mt + 1) * 128, oh * 1024 : (oh + 1) * 1024],
                    ost[:, oh * 1024 : (oh + 1) * 1024],
                )
                if oh == 1:
                    del ost_live[(b, mt)]
            elif oh == 1:
                nc.sync.dma_start(out[b, mt * 128 : (mt + 1) * 128, :], ost[:])
                del ost_live[(b, mt)]

        # ---- attention unit with woven filler chunks ----
        def emit_unit(b, g, early=(), mid=(), post=()):
            """S^T+exp for (b,g), SV lagged, DVE denom tree, fillers woven.

            PE stream: S0 S1 [early] S2 S3 S4+sv0 S5+sv1 S6+sv2 S7+sv3
                       [mid] sv4..sv7 den0 den1, recip+svmul (DVE), [post].
            DVE stream (per unit): t0..t3 (pair sums of exp tiles, bf16 2x),
            u0 u1 acc (tree), recip+svmul. The mid fillers buy time for
            exp7 + the tree tail before the PE hits den0.
            """
            pT = [ptp.tile([128, T], BF, tag=f"p{j}", name=f"pT{j}_{g}_{b}") for j in range(TT)]
            pv2 = pvp.tile([128, T], F32, tag="pv", name=f"pssv_{g}_{b}")
            tt = [trp.tile([128, T], BF, tag=f"t{i}", name=f"t{i}_{g}_{b}") for i in range(4)]
            uu = [trp.tile([128, T], BF, tag=f"u{i}", name=f"u{i}_{g}_{b}") for i in range(2)]
            acc = accp.tile([128, T], BF, tag="acc", name=f"acc_{g}_{b}")

            def s_step(j):
                # one 2-bank PSUM tile per j: both mch matmuls land in it,
                # then a single 1024-wide exp evacuates it (halves ACT's
                # per-instruction init overhead vs two 512-wide exps)
                ps = spsp.tile([128, T], F32, tag="sps", name=f"pss{j}_{g}_{b}")
                for mch in range(MCH):
                    nc.tensor.matmul(
                        ps[:, mch * 512 : (mch + 1) * 512],
                        kT_s[b][g][:, j * 128 : (j + 1) * 128],
                        qT_s[b][g][:, mch * 512 : (mch + 1) * 512],
                        start=True,
                        stop=True,
                    )
                nc.scalar.activation(pT[j][:], ps[:], AF.Exp)

            def sv_step(j):
                for mch in range(MCH):
                    nc.tensor.matmul(
                        pv2[:, mch * 512 : (mch + 1) * 512],
                        v_s[b][j][:, g * 128 : (g + 1) * 128],
                        pT[j][:, mch * 512 : (mch + 1) * 512],
                        start=(j == 0),
                        stop=(j == TT - 1),
                    )

            s_step(0)
            s_step(1)
            nc.vector.tensor_add(tt[0][:], pT[0][:], pT[1][:])
            s_step(2)
            for f in early:
                f()
            s_step(3)
            nc.vector.tensor_add(tt[1][:], pT[2][:], pT[3][:])
            nc.vector.tensor_add(uu[0][:], tt[0][:], tt[1][:])
            for j in range(4, TT):
                s_step(j)
                sv_step(j - 4)
                if j == 5:
                    nc.vector.tensor_add(tt[2][:], pT[4][:], pT[5][:])
                if j == 7:
                    nc.vector.tensor_add(tt[3][:], pT[6][:], pT[7][:])
                    nc.vector.tensor_add(uu[1][:], tt[2][:], tt[3][:])
                    nc.vector.tensor_add(acc[:], uu[0][:], uu[1][:])
            for f in mid:
                f()
            for j in range(TT - 4, TT):
                sv_step(j)
            # denominator partition-reduction: on the (otherwise idle) Pool
            # engine for most units; the last unit keeps the PE ones-matmul
            # because its denom→recip→svmul chain gates the whole b1-proj
            # epilogue and the Q7 software op's latency is less predictable
            # denominator partition-reduction: on the (otherwise idle) Pool
            # engine for most units; the last unit keeps the PE ones-matmul
            # because its denom→recip→svmul chain gates the whole b1-proj
            # epilogue and the Q7 software op's latency is less predictable
            if (b, g) != (1, 3):
                pd = rdnp.tile([128, T], F32, tag="pd", name=f"pd{g}_{b}")
                nc.gpsimd.partition_all_reduce(
                    pd[:], acc[:], channels=128, reduce_op=bass_isa.ReduceOp.add
                )
            else:
                pd = spsp.tile([128, T], F32, tag="sps", name=f"psd_{g}_{b}")
                for m in range(MCH):
                    nc.tensor.matmul(
                        pd[:, m * 512 : (m + 1) * 512],
                        ones_s[:],
                        acc[:, m * 512 : (m + 1) * 512],
                        start=True,
                        stop=True,
                    )
            rden = rdnp.tile([128, T], F32, tag="rd", name=f"rden{g}_{b}")
            nc.vector.reciprocal(rden[:], pd[:])
            nc.vector.tensor_mul(svT[b][g][:], pv2[:], rden[:])
            for f in post:
                f()

        # ---- whole-kernel schedule ----
        # prologue: batch-0 QKV — all Q first (its inputs stream in first),
        # then K, then V
        chunk_q(0, 0, ct_outer=True)
        for g in range(1, G):
            chunk_q(0, g)
        for g in range(G):
            chunk_k(0, g)
        for jj in range(TT // 2):
            chunk_v(0, jj)

        # batch-0 units carry batch-1 QKV as filler;
        # batch-1 units carry batch-0 proj as filler.
        def fq(g):
            return lambda: chunk_q(1, g)

        def fk(g):
            return lambda: chunk_k(1, g)

        def fv(jj):
            return lambda: chunk_v(1, jj)

        def fp(b, a, evac_act=False):
            return lambda: chunk_proj(b, a // 2, a % 2, evac_act=evac_act)

        emit_unit(0, 0, early=[fq(0)], mid=[fq(1)], post=[fq(2)])
        emit_unit(0, 1, early=[fq(3)], mid=[fk(0)], post=[fk(1)])
        emit_unit(0, 2, early=[fk(2)], mid=[fk(3)], post=[fv(0)])
        emit_unit(0, 3, early=[fv(1)], mid=[fv(2)], post=[fv(3)])
        # proj(0) needs svT(0,3), which lands during unit (1,0): no early
        # there. unit (1,3) gets a multi-chunk post so the PE has work
        # while its own DVE tail (tree→den→recip→svmul for svT(1,3))
        # retires — the entire b1 proj epilogue waits on that tail.
        emit_unit(1, 0, mid=[fp(0, 0), fp(0, 1)], post=[fp(0, 2)])
        emit_unit(1, 1, early=[fp(0, 3)], mid=[fp(0, 4)], post=[fp(0, 5)])
        emit_unit(1, 2, early=[fp(0, 6), fp(0, 7)], mid=[fp(0, 8)], post=[fp(0, 9)])
        # unit (1,3): filler evacs on ACT — its DVE chain (tree → recip →
        # svmul for svT(1,3)) gates the entire b1 proj epilogue
        emit_unit(1, 3, early=[fp(0, 10, True), fp(0, 11, True)],
                  mid=[fp(0, 12, True)],
                  post=[fp(0, 13, True), fp(0, 14, True), fp(0, 15, True)])
        # epilogue: batch-1 proj
        for a in range(16):
            chunk_proj(1, a // 2, a % 2)


def _build(has_cout):
    nc = bacc.Bacc(
        "TRN2", target_bir_lowering=False, debug=False, num_devices=N_CORES
    )
    with tile.TileContext(nc) as tc:
        _emit(tc, has_cout)
    nc.compile()
    return nc


def get_nc(has_cout=False):
    key = ("nc", has_cout)
    if key not in _CACHE:
        _CACHE[key] = _build(has_cout)
    return _CACHE[key]


def prep_inputs(attention, op_param, q_w, q_b, k_w, k_b, v_w, v_b, proj_w, proj_b):
    """Host-side layout prep: fold scales, transpose, cast to bf16, shard."""
    bf16 = ml_dtypes.bfloat16
    f32 = np.float32

    att = np.asarray(attention, f32)
    op = np.asarray(op_param, f32)

    # (n, t, c) -> (n, c, t), bf16
    xt_att = np.ascontiguousarray(att.transpose(0, 2, 1)).astype(bf16)
    xt_op = np.ascontiguousarray(op.transpose(0, 2, 1)).astype(bf16)

    wqT = np.ascontiguousarray(
        (np.asarray(q_w, f32) * (EQ_SCALE * ATTN_SCALE)).T
    ).astype(bf16)
    wkT = np.ascontiguousarray((np.asarray(k_w, f32) * EQ_SCALE).T).astype(bf16)
    wvT = np.ascontiguousarray((np.asarray(v_w, f32) * EQ_SCALE).T).astype(bf16)
    # proj is also an EqualLinear: weight scale 1/sqrt(PLANES) = EQ_SCALE
    wp_scaled = np.asarray(proj_w, f32) * EQ_SCALE
    wpT = np.ascontiguousarray(wp_scaled.T).astype(bf16)

    qb2 = (np.asarray(q_b, f32) * ATTN_SCALE).reshape(P, 1)
    kb2 = np.asarray(k_b, f32).reshape(P, 1)
    # sum_l sim = 1, so v_b contributes proj_w @ v_b to every output row
    cout = wp_scaled @ np.asarray(v_b, f32) + np.asarray(proj_b, f32)
    has_cout = bool(np.any(cout != 0.0))
    coutb = np.ascontiguousarray(np.broadcast_to(cout[None, :], (128, O))).astype(f32)

    in_maps = []
    for core in range(N_CORES):
        lo, hi = core * B, (core + 1) * B
        m = {
            "xt_op": np.ascontiguousarray(xt_op[lo:hi]),
            "xt_att": np.ascontiguousarray(xt_att[lo:hi]),
            "wqT": wqT,
            "wkT": wkT,
            "wvT": wvT,
            "wpT": wpT,
            "qb": qb2,
            "kb": kb2,
        }
        if has_cout:
            m["coutb"] = coutb
        in_maps.append(m)
    return in_maps


def run(in_maps, trace=False, **kw):
    has_cout = "coutb" in in_maps[0]
    nc = get_nc(has_cout)
    res = run_bass_kernel_spmd(nc, in_maps, list(range(N_CORES)), trace=trace, **kw)
    return res


def kernel(**inputs) -> np.ndarray:
    in_maps = prep_inputs(**inputs)
    res = run(in_maps)
    out = np.concatenate([res.results[i]["out"] for i in range(N_CORES)], axis=0)
    return out.astype(np.float32)


# revision 83
# speedup vs baseline: 1.0038x; 1.0016x over previous
"""Trainium2 Bass kernel for nn_Attention_81793357185069.

4-group attention: N=16, L=M=1024, in/param dim 512, planes 512, out 2048.
Strategy: data-parallel over batch N across 8 NeuronCores (2 batches/core),
zero collectives. All matmuls run in bf16 (1 cycle/row on PE vs 4 for fp32)
with fp32 PSUM accumulation.

Layout trick: everything is computed in "transposed" space so the PE
contraction dim always lands on partitions with zero on-chip transposes:
  - host pre-transposes activations X -> X^T (c, t) and weights W -> W^T
  - Q^T/K^T (planes, t) and V (t, planes) come straight out of projections
  - scores S^T (l, m) = (K^T_g).T @ Q^T_g per group
  - softmax has no max-subtraction (scores are bounded ~ N(0, 0.5^2)) and
    normalization is deferred: P = exp(S^T), denominator j-tiles are
    tree-summed on DVE (bf16, 2x mode) and the final 128-partition
    reduction runs on the otherwise-idle Pool/GPSIMD engine
    (partition_all_reduce) — zero PE matmuls for the denominator in 7 of
    8 units (the last unit keeps a 2-matmul ones-reduction on PE because
    its denom chain gates the b1-proj epilogue)
  - sv (d, m) = V_g.T @ P accumulated over l-tiles, scaled by 1/denom on
    PSUM evacuation
  - out (m, o) = sv_all.T @ Wproj^T; the scale 1/sqrt(512), ATTN_SCALE and
    all biases are folded in host-side (v_b/proj_b fold into a constant
    output vector since sum_l sim = 1).

Scheduling: each (batch, group) attention unit is woven with "filler"
matmul work — the next batch's QKV projections or the previous batch's
output projection — so the in-order PE stream never stalls on ACT exp
throughput or on the DVE denominator tree (the mid fillers sit between
the last S matmul and the SV tail, buying time for the tree before the
reduction consumes it). Engine balance: ACT carries all exps (one
1024-wide exp per S j-step from a 2-bank PSUM tile) + prologue/epilogue
evacs; DVE carries the denominator tree, recip/svmul, and the
woven-filler evacs; Pool carries the denominator partition-reductions.
DMAs are few and large (one 3-dim-AP DMA per input tensor via
AP.rearrange; one merged fp16 output DMA per 128-token row-block)
because per-DMA queue overhead (~1us) dominates with many small
transfers. ~3us of dummy PE matmuls during the initial DMA wait keep
the HAM clock-gate warm for the first real matmuls.
"""

import math

import ml_dtypes
import numpy as np

import concourse.bass as bass
import concourse.mybir as mybir
import concourse.tile as tile
from concourse import bacc, bass_isa
from concourse.bass_utils import run_bass_kernel_spmd

N_CORES = 8
N = 16
B = N // N_CORES  # batches per core
T = 1024  # L == M
C = 512  # in/param dim
P = 512  # planes
O = 2048  # out dim
G = 4  # groups
D = P // G  # 128 group planes
ATTN_SCALE = P ** (-0.5)
EQ_SCALE = 1.0 / math.sqrt(C)

CT = C // 128  # 4 contraction tiles
TT = T // 128  # 8 l/m tiles
MCH = T // 512  # 2 moving chunks of 512

BF = mybir.dt.bfloat16
F16 = mybir.dt.float16
F32 = mybir.dt.float32

_CACHE: dict = {}


def _emit(tc, has_cout):
    nc = tc.nc
    AF = mybir.ActivationFunctionType

    xt_op = nc.dram_tensor("xt_op", [B, C, T], BF, kind="ExternalInput").ap()
    xt_att = nc.dram_tensor("xt_att", [B, C, T], BF, kind="ExternalInput").ap()
    wqT = nc.dram_tensor("wqT", [C, P], BF, kind="ExternalInput").ap()
    wkT = nc.dram_tensor("wkT", [C, P], BF, kind="ExternalInput").ap()
    wvT = nc.dram_tensor("wvT", [C, P], BF, kind="ExternalInput").ap()
    wpT = nc.dram_tensor("wpT", [P, O], BF, kind="ExternalInput").ap()
    qb = nc.dram_tensor("qb", [P, 1], F32, kind="ExternalInput").ap()
    kb = nc.dram_tensor("kb", [P, 1], F32, kind="ExternalInput").ap()
    coutb = (
        nc.dram_tensor("coutb", [128, O], F32, kind="ExternalInput").ap()
        if has_cout
        else None
    )
    # fp16 device output: halves output-DMA bytes (the epilogue is
    # DMA-rate-limited with fp32), host casts back to fp32; adds ~1e-4
    # relative error
    out = nc.dram_tensor("out", [B, T, O], F16, kind="ExternalOutput").ap()

    with (
        tc.tile_pool(name="const", bufs=1) as const,
        tc.tile_pool(name="xt", bufs=1) as xtp,
        tc.tile_pool(name="qkv", bufs=2) as qkvp,
        tc.tile_pool(name="pt", bufs=2) as ptp,
        tc.tile_pool(name="tr", bufs=1) as trp,
        tc.tile_pool(name="accp", bufs=2) as accp,
        tc.tile_pool(name="rdn", bufs=2) as rdnp,
        tc.tile_pool(name="svt", bufs=2) as svtp,
        tc.tile_pool(name="ost", bufs=2) as ostp,
        tc.tile_pool(name="ps", bufs=2, space="PSUM") as psp,
        tc.tile_pool(name="sps", bufs=2, space="PSUM") as spsp,
        tc.tile_pool(name="pvp", bufs=1, space="PSUM") as pvp,
    ):
        # ---- constants / weights (one merged tile per tensor) ----
        # weight layout: [128, ct * P] — ct-tile i at columns [i*P, (i+1)*P)
        wq_s = const.tile([128, CT * P], BF, tag="wq", name="wq")
        wk_s = const.tile([128, CT * P], BF, tag="wk", name="wk")
        wv_s = const.tile([128, CT * P], BF, tag="wv", name="wv")
        wp_s = const.tile([128, CT * O], BF, tag="wp", name="wp")
        qb_s = const.tile([128, G], F32, tag="qb", name="qb")
        kb_s = const.tile([128, G], F32, tag="kb", name="kb")
        cout_s = const.tile([128, O], F32, tag="cout", name="cout") if has_cout else None
        ones_s = const.tile([128, 128], BF, tag="ones", name="ones")

        # activations: [128, ct * T]
        xo_s = {b: xtp.tile([128, CT * T], BF, tag=f"xo{b}", name=f"xo_{b}") for b in range(B)}
        xa_s = {b: xtp.tile([128, CT * T], BF, tag=f"xa{b}", name=f"xa_{b}") for b in range(B)}
        qT_s = {b: [qkvp.tile([128, T], BF, tag=f"q{g}", name=f"qT{g}_{b}") for g in range(G)] for b in range(B)}
        kT_s = {b: [qkvp.tile([128, T], BF, tag=f"k{g}", name=f"kT{g}_{b}") for g in range(G)] for b in range(B)}
        v_s = {b: [qkvp.tile([128, P], BF, tag=f"v{j}", name=f"v{j}_{b}") for j in range(TT)] for b in range(B)}
        svT = {b: [svtp.tile([128, T], BF, tag=f"s{g}", name=f"svT{g}_{b}") for g in range(G)] for b in range(B)}

        # ---- input DMAs: one per tensor, need-ordered, spread over both
        # HWDGE queues (scalar + sync) so bandwidth serves the earliest
        # consumer. Per-DMA queue overhead is ~1us, so few+large wins.
        xo0_src = xt_op[0].rearrange("(ct p) t -> p ct t", p=128)
        xa0_src = xt_att[0].rearrange("(ct p) t -> p ct t", p=128)
        # per-ct xo DMAs so the ct-outer first Q chunk can trickle-start
        # as tiles land; everything else one DMA per tensor in need order
        for i in range(CT):
            nc.scalar.dma_start(
                xo_s[0][:, i * T : (i + 1) * T], xo0_src[:, i : i + 1]
            )
        nc.sync.dma_start(wq_s[:], wqT.rearrange("(ct p) o -> p ct o", p=128))
        nc.sync.dma_start(qb_s[:], qb.rearrange("(g p) one -> p (g one)", p=128))
        nc.sync.dma_start(wk_s[:], wkT.rearrange("(ct p) o -> p ct o", p=128))
        nc.scalar.dma_start(xa_s[0][:, : 2 * T], xa0_src[:, :2])
        nc.scalar.dma_start(xa_s[0][:, 2 * T :], xa0_src[:, 2:])
        nc.sync.dma_start(kb_s[:], kb.rearrange("(g p) one -> p (g one)", p=128))
        nc.sync.dma_start(wv_s[:], wvT.rearrange("(ct p) o -> p ct o", p=128))
        nc.vector.memset(ones_s[:], 1.0)
        # preload the ACT exp table set during the idle head (the implicit
        # table load would otherwise stall the first critical exp by ~1.3us)
        expwarm = const.tile([128, 1], F32, tag="expwarm", name="expwarm")
        nc.scalar.activation(expwarm[:], ones_s[:, :1], AF.Exp)
        # PE warmup: ~3us of dummy matmuls during the initial DMA wait so
        # the HAM clock-gate (and the cost model's p-state ramp) reaches
        # full speed before the first real matmul; the result is never read
        warm = psp.tile([128, 512], F32, tag="ps", name="warm")
        for i in range(28):
            nc.tensor.matmul(
                warm[:, :128], ones_s[:], ones_s[:],
                start=(i == 0), stop=(i == 27),
            )
        nc.scalar.dma_start(
            xo_s[1][:], xt_op[1].rearrange("(ct p) t -> p ct t", p=128)
        )
        nc.scalar.dma_start(
            xa_s[1][:], xt_att[1].rearrange("(ct p) t -> p ct t", p=128)
        )
        nc.sync.dma_start(wp_s[:], wpT.rearrange("(ct p) o -> p ct o", p=128))
        if has_cout:
            nc.sync.dma_start(cout_s[:], coutb[:, :])

        # ---- chunk emitters (each: one PSUM group = 8 MMs + evacuation).
        # Evac engine rule: batch-0 QKV runs in the prologue (ACT idle →
        # ACT); batch-1 QKV is woven into b0 units whose ACT is saturated
        # with exps (→ DVE). b0 proj is woven into b1 units (→ DVE); b1
        # proj runs in the epilogue (→ ACT).
        def chunk_q(b, g, ct_outer=False):
            # ct_outer: first prologue chunk only — lets matmuls start as
            # soon as the first ct tile of xo lands (startup trickle)
            if ct_outer:
                pq = [
                    psp.tile([128, 512], F32, tag="ps", name=f"psq{g}{mch}_{b}")
                    for mch in range(MCH)
                ]
                for ct in range(CT):
                    for mch in range(MCH):
                        nc.tensor.matmul(
                            pq[mch][:],
                            wq_s[:, ct * P + g * 128 : ct * P + (g + 1) * 128],
                            xo_s[b][:, ct * T + mch * 512 : ct * T + (mch + 1) * 512],
                            start=(ct == 0),
                            stop=(ct == CT - 1),
                        )
                for mch in range(MCH):
                    osl = qT_s[b][g][:, mch * 512 : (mch + 1) * 512]
                    nc.scalar.activation(
                        osl, pq[mch][:], AF.Identity, bias=qb_s[:, g : g + 1]
                    )
                return
            for mch in range(MCH):
                pq = psp.tile([128, 512], F32, tag="ps", name=f"psq{g}{mch}_{b}")
                for ct in range(CT):
                    nc.tensor.matmul(
                        pq[:],
                        wq_s[:, ct * P + g * 128 : ct * P + (g + 1) * 128],
                        xo_s[b][:, ct * T + mch * 512 : ct * T + (mch + 1) * 512],
                        start=(ct == 0),
                        stop=(ct == CT - 1),
                    )
                osl = qT_s[b][g][:, mch * 512 : (mch + 1) * 512]
                if b == 0:
                    nc.scalar.activation(
                        osl, pq[:], AF.Identity, bias=qb_s[:, g : g + 1]
                    )
                else:
                    nc.vector.tensor_scalar_add(osl, pq[:], qb_s[:, g : g + 1])

        def chunk_k(b, g):
            for mch in range(MCH):
                pk = psp.tile([128, 512], F32, tag="ps", name=f"psk{g}{mch}_{b}")
                for ct in range(CT):
                    nc.tensor.matmul(
                        pk[:],
                        wk_s[:, ct * P + g * 128 : ct * P + (g + 1) * 128],
                        xa_s[b][:, ct * T + mch * 512 : ct * T + (mch + 1) * 512],
                        start=(ct == 0),
                        stop=(ct == CT - 1),
                    )
                osl = kT_s[b][g][:, mch * 512 : (mch + 1) * 512]
                if b == 0:
                    nc.scalar.activation(
                        osl, pk[:], AF.Identity, bias=kb_s[:, g : g + 1]
                    )
                else:
                    nc.vector.tensor_scalar_add(osl, pk[:], kb_s[:, g : g + 1])

        def chunk_v(b, jj):
            for jh in range(2):
                j = jj * 2 + jh
                pv = psp.tile([128, 512], F32, tag="ps", name=f"psv{j}_{b}")
                for ct in range(CT):
                    nc.tensor.matmul(
                        pv[:],
                        xa_s[b][:, ct * T + j * 128 : ct * T + (j + 1) * 128],
                        wv_s[:, ct * P : (ct + 1) * P],
                        start=(ct == 0),
                        stop=(ct == CT - 1),
                    )
                if b == 0:
                    nc.scalar.copy(v_s[b][j][:], pv[:])
                else:
                    nc.vector.tensor_copy(v_s[b][j][:], pv[:])

        ost_live = {}

        def chunk_proj(b, mt, oh, evac_act=False):
            # ost holds both oh halves of an mt row-block; one merged
            # output DMA per mt fires when the oh=1 half is evacuated
            if oh == 0:
                ost_live[(b, mt)] = ostp.tile(
                    [128, O], F16, tag="ost", name=f"ost{mt}_{b}"
                )
            ost = ost_live[(b, mt)]
            for oc in range(2):
                po = psp.tile([128, 512], F32, tag="ps", name=f"pso{mt}{oh}{oc}_{b}")
                for g in range(G):
                    nc.tensor.matmul(
                        po[:],
                        svT[b][g][:, mt * 128 : (mt + 1) * 128],
                        wp_s[:, g * O + oh * 1024 + oc * 512 : g * O + oh * 1024 + (oc + 1) * 512],
                        start=(g == 0),
                        stop=(g == G - 1),
                    )
                osl = ost[:, oh * 1024 + oc * 512 : oh * 1024 + (oc + 1) * 512]
                if has_cout:
                    nc.vector.tensor_add(
                        osl, po[:],
                        cout_s[:, oh * 1024 + oc * 512 : oh * 1024 + (oc + 1) * 512],
                    )
                elif b == 0 and not evac_act:
                    nc.vector.tensor_copy(osl, po[:])
                else:
                    nc.scalar.copy(osl, po[:])
                if b == 1 and mt == TT - 1:
                    # last row-block: fire per-oc quarters right after each
                    # evac so the final DMA (and the kernel tail behind it)
                    # is as short as possible
                    lo = oh * 1024 + oc * 512
                    nc.sync.dma_start(
                        out[b, mt * 128 : (mt + 1) * 128, lo : lo + 512],
                        ost[:, lo : lo + 512],
                    )
            if b == 1 and mt == TT - 1:
                if oh == 1:
                    del ost_live[(b, mt)]
            elif oh == 1:
                nc.sync.dma_start(out[b, mt * 128 : (mt + 1) * 128, :], ost[:])
                del ost_live[(b, mt)]

        # ---- attention unit with woven filler chunks ----
        def emit_unit(b, g, early=(), mid=(), post=()):
            """S^T+exp for (b,g), SV lagged, DVE denom tree, fillers woven.

            PE stream: S0 S1 [early] S2 S3 S4+sv0 S5+sv1 S6+sv2 S7+sv3
                       [mid] sv4..sv7 den0 den1, recip+svmul (DVE), [post].
            DVE stream (per unit): t0..t3 (pair sums of exp tiles, bf16 2x),
            u0 u1 acc (tree), recip+svmul. The mid fillers buy time for
            exp7 + the tree tail before the PE hits den0.
            """
            pT = [ptp.tile([128, T], BF, tag=f"p{j}", name=f"pT{j}_{g}_{b}") for j in range(TT)]
            pv2 = pvp.tile([128, T], F32, tag="pv", name=f"pssv_{g}_{b}")
            tt = [trp.tile([128, T], BF, tag=f"t{i}", name=f"t{i}_{g}_{b}") for i in range(4)]
            uu = [trp.tile([128, T], BF, tag=f"u{i}", name=f"u{i}_{g}_{b}") for i in range(2)]
            acc = accp.tile([128, T], BF, tag="acc", name=f"acc_{g}_{b}")

            def s_step(j):
                # one 2-bank PSUM tile per j: both mch matmuls land in it,
                # then a single 1024-wide exp evacuates it (halves ACT's
                # per-instruction init overhead vs two 512-wide exps)
                ps = spsp.tile([128, T], F32, tag="sps", name=f"pss{j}_{g}_{b}")
                for mch in range(MCH):
                    nc.tensor.matmul(
                        ps[:, mch * 512 : (mch + 1) * 512],
                        kT_s[b][g][:, j * 128 : (j + 1) * 128],
                        qT_s[b][g][:, mch * 512 : (mch + 1) * 512],
                        start=True,
                        stop=True,
                    )
                nc.scalar.activation(pT[j][:], ps[:], AF.Exp)

            def sv_step(j):
                for mch in range(MCH):
                    nc.tensor.matmul(
                        pv2[:, mch * 512 : (mch + 1) * 512],
                        v_s[b][j][:, g * 128 : (g + 1) * 128],
                        pT[j][:, mch * 512 : (mch + 1) * 512],
                        start=(j == 0),
                        stop=(j == TT - 1),
                    )

            s_step(0)
            s_step(1)
            nc.vector.tensor_add(tt[0][:], pT[0][:], pT[1][:])
            s_step(2)
            for f in early:
                f()
            s_step(3)
            nc.vector.tensor_add(tt[1][:], pT[2][:], pT[3][:])
            nc.vector.tensor_add(uu[0][:], tt[0][:], tt[1][:])
            for j in range(4, TT):
                s_step(j)
                sv_step(j - 4)
                if j == 5:
                    nc.vector.tensor_add(tt[2][:], pT[4][:], pT[5][:])
                if j == 7:
                    nc.vector.tensor_add(tt[3][:], pT[6][:], pT[7][:])
                    nc.vector.tensor_add(uu[1][:], tt[2][:], tt[3][:])
                    nc.vector.tensor_add(acc[:], uu[0][:], uu[1][:])
            for f in mid:
                f()
            for j in range(TT - 4, TT):
                sv_step(j)
            # denominator partition-reduction: on the (otherwise idle) Pool
            # engine for most units; the last unit keeps the PE ones-matmul
            # because its denom→recip→svmul chain gates the whole b1-proj
            # epilogue and the Q7 software op's latency is less predictable
            # denominator partition-reduction: on the (otherwise idle) Pool
            # engine for most units; the last unit keeps the PE ones-matmul
            # because its denom→recip→svmul chain gates the whole b1-proj
            # epilogue and the Q7 software op's latency is less predictable
            if (b, g) != (1, 3):
                pd = rdnp.tile([128, T], F32, tag="pd", name=f"pd{g}_{b}")
                nc.gpsimd.partition_all_reduce(
                    pd[:], acc[:], channels=128, reduce_op=bass_isa.ReduceOp.add
                )
            else:
                pd = spsp.tile([128, T], F32, tag="sps", name=f"psd_{g}_{b}")
                for m in range(MCH):
                    nc.tensor.matmul(
                        pd[:, m * 512 : (m + 1) * 512],
                        ones_s[:],
                        acc[:, m * 512 : (m + 1) * 512],
                        start=True,
                        stop=True,
                    )
            rden = rdnp.tile([128, T], F32, tag="rd", name=f"rden{g}_{b}")
            nc.vector.reciprocal(rden[:], pd[:])
            nc.vector.tensor_mul(svT[b][g][:], pv2[:], rden[:])
            for f in post:
                f()

        # ---- whole-kernel schedule ----
        # prologue: batch-0 QKV — all Q first (its inputs stream in first),
        # then K, then V
        chunk_q(0, 0, ct_outer=True)
        for g in range(1, G):
            chunk_q(0, g)
        for g in range(G):
            chunk_k(0, g)
        for jj in range(TT // 2):
            chunk_v(0, jj)

        # batch-0 units carry batch-1 QKV as filler;
        # batch-1 units carry batch-0 proj as filler.
        def fq(g):
            return lambda: chunk_q(1, g)

        def fk(g):
            return lambda: chunk_k(1, g)

        def fv(jj):
            return lambda: chunk_v(1, jj)

        def fp(b, a, evac_act=False):
            return lambda: chunk_proj(b, a // 2, a % 2, evac_act=evac_act)

        emit_unit(0, 0, early=[fq(0)], mid=[fq(1)], post=[fq(2)])
        emit_unit(0, 1, early=[fq(3)], mid=[fk(0)], post=[fk(1)])
        emit_unit(0, 2, early=[fk(2)], mid=[fk(3)], post=[fv(0)])
        emit_unit(0, 3, early=[fv(1)], mid=[fv(2)], post=[fv(3)])
        # proj(0) needs svT(0,3), which lands during unit (1,0): no early
        # there. unit (1,3) gets a multi-chunk post so the PE has work
        # while its own DVE tail (tree→den→recip→svmul for svT(1,3))
        # retires — the entire b1 proj epilogue waits on that tail.
        emit_unit(1, 0, mid=[fp(0, 0), fp(0, 1)], post=[fp(0, 2)])
        emit_unit(1, 1, early=[fp(0, 3)], mid=[fp(0, 4)], post=[fp(0, 5)])
        emit_unit(1, 2, early=[fp(0, 6), fp(0, 7)], mid=[fp(0, 8)], post=[fp(0, 9)])
        # unit (1,3): filler evacs on ACT — its DVE chain (tree → recip →
        # svmul for svT(1,3)) gates the entire b1 proj epilogue
        emit_unit(1, 3, early=[fp(0, 10, True), fp(0, 11, True)],
                  mid=[fp(0, 12, True)],
                  post=[fp(0, 13, True), fp(0, 14, True), fp(0, 15, True)])
        # epilogue: batch-1 proj
        for a in range(16):
            chunk_proj(1, a // 2, a % 2)


def _build(has_cout):
    nc = bacc.Bacc(
        "TRN2", target_bir_lowering=False, debug=False, num_devices=N_CORES
    )
    with tile.TileContext(nc) as tc:
        _emit(tc, has_cout)
    nc.compile()
    return nc


def get_nc(has_cout=False):
    key = ("nc", has_cout)
    if key not in _CACHE:
        _CACHE[key] = _build(has_cout)
    return _CACHE[key]


def prep_inputs(attention, op_param, q_w, q_b, k_w, k_b, v_w, v_b, proj_w, proj_b):
    """Host-side layout prep: fold scales, transpose, cast to bf16, shard."""
    bf16 = ml_dtypes.bfloat16
    f32 = np.float32

    att = np.asarray(attention, f32)
    op = np.asarray(op_param, f32)

    # (n, t, c) -> (n, c, t), bf16
    xt_att = np.ascontiguousarray(att.transpose(0, 2, 1)).astype(bf16)
    xt_op = np.ascontiguousarray(op.transpose(0, 2, 1)).astype(bf16)

    wqT = np.ascontiguousarray(
        (np.asarray(q_w, f32) * (EQ_SCALE * ATTN_SCALE)).T
    ).astype(bf16)
    wkT = np.ascontiguousarray((np.asarray(k_w, f32) * EQ_SCALE).T).astype(bf16)
    wvT = np.ascontiguousarray((np.asarray(v_w, f32) * EQ_SCALE).T).astype(bf16)
    # proj is also an EqualLinear: weight scale 1/sqrt(PLANES) = EQ_SCALE
    wp_scaled = np.asarray(proj_w, f32) * EQ_SCALE
    wpT = np.ascontiguousarray(wp_scaled.T).astype(bf16)

    qb2 = (np.asarray(q_b, f32) * ATTN_SCALE).reshape(P, 1)
    kb2 = np.asarray(k_b, f32).reshape(P, 1)
    # sum_l sim = 1, so v_b contributes proj_w @ v_b to every output row
    cout = wp_scaled @ np.asarray(v_b, f32) + np.asarray(proj_b, f32)
    has_cout = bool(np.any(cout != 0.0))
    coutb = np.ascontiguousarray(np.broadcast_to(cout[None, :], (128, O))).astype(f32)

    in_maps = []
    for core in range(N_CORES):
        lo, hi = core * B, (core + 1) * B
        m = {
            "xt_op": np.ascontiguousarray(xt_op[lo:hi]),
            "xt_att": np.ascontiguousarray(xt_att[lo:hi]),
            "wqT": wqT,
            "wkT": wkT,
            "wvT": wvT,
            "wpT": wpT,
            "qb": qb2,
            "kb": kb2,
        }
        if has_cout:
            m["coutb"] = coutb
        in_maps.append(m)
    return in_maps


def run(in_maps, trace=False, **kw):
    has_cout = "coutb" in in_maps[0]
    nc = get_nc(has_cout)
    res = run_bass_kernel_spmd(nc, in_maps, list(range(N_CORES)), trace=trace, **kw)
    return res


def kernel(**inputs) -> np.ndarray:
    in_maps = prep_inputs(**inputs)
    res = run(in_maps)
    out = np.concatenate([res.results[i]["out"] for i in range(N_CORES)], axis=0)
    return out.astype(np.float32)
